# revision 5
# baseline (speedup 1.0000x reference)
"""BindingPocketGNN (3-layer GCN, N=50000, E=800000) on 8 Trainium2 NeuronCores.

Distribution: nodes sharded into 8 contiguous ranges (6250/core). Each core owns the
scatter/aggregation for its destination-node range; edges are routed (host-side) to the
core owning their destination. Source features are gathered from a replicated
node-major table (input x for layer 1; AllGather-replicated activations for layers 2/3).

Per layer, on each core (feat-major formulation so BN/bias are per-partition):
    z^T[f_in, d]  = sum_e  msg_e[f_in] * mask[e, d]      (TensorE: msg.T @ (iota==dst) mask)
    zs            = z^T * dinv[dst]                      (fused in PSUM->SBUF copy)
    y^T[f_out, d] = W.T @ zs                             (TensorE)
    stats         = AllReduce(sum/sumsq of y)            (1KB collective; BN layers)
    act^T         = Relu(A*y^T + B)                      (ScalarE, per-partition A/B)
    h             = act^T transposed to node-major       (TensorE transpose)
    table_{l+1}   = AllGather(h * ...)                   (collective; layers 1,2)
Layer 3 feeds a [128->1] FC matmul; +fcb and sigmoid applied on host.

deg/dinv and dinv[src] pre-scaling of x are computed on host (pure input transforms);
per-edge norm = dinv[src]*dinv[dst] is realized as table-prescale x dinv_bcast.
"""
import sys
if "/opt/trn_rl_repo" not in sys.path:
    sys.path.insert(0, "/opt/trn_rl_repo")

import numpy as np

import concourse.bass as bass
import concourse.bacc as bacc
import concourse.mybir as mybir
import concourse.tile as tile
from concourse import bass_utils
from concourse.masks import make_identity

N = 50000
E = 800000
IN, HID = 64, 128
BN_EPS = 1e-5
NCORES = 8
NPC = N // NCORES          # 6250 nodes per core
P = 128
NT = (NPC + P - 1) // P    # 49 dst tiles per core
LAST_D = NPC - (NT - 1) * P  # 106

BF16_TABLES = True         # gather tables + masks in bf16 (fp32 accumulation in PSUM)

F32 = mybir.dt.float32
I32 = mybir.dt.int32
BF16 = mybir.dt.bfloat16
DT_TAB = BF16 if BF16_TABLES else F32
NP_TAB = np.dtype("bfloat16") if False else None  # numpy bf16 via ml_dtypes below

import ml_dtypes
NP_TAB = np.dtype(ml_dtypes.bfloat16) if BF16_TABLES else np.dtype(np.float32)
import os
STAGE = int(os.environ.get("GCN_STAGE", "6"))

Alu = mybir.AluOpType
Act = mybir.ActivationFunctionType

_NC_CACHE = {}


def _build(T, S):
    """Build+schedule the SPMD program. T = total edge subtiles, S = list of subtile
    counts per dst tile (len NT, sum T). Identical for all 8 cores."""
    nc = bacc.Bacc("TRN2", target_bir_lowering=False, debug=False, num_devices=NCORES)

    # ---- I/O ----
    xs = nc.dram_tensor("xs", [N, IN], DT_TAB, kind="ExternalInput")
    gidx_d = nc.dram_tensor("gidx", [P, T], I32, kind="ExternalInput")
    dloc_d = nc.dram_tensor("dloc", [P, T], F32, kind="ExternalInput")
    dinv_d = nc.dram_tensor("dinv_sl", [P, NT], F32, kind="ExternalInput")
    W_d = [
        nc.dram_tensor("W1", [IN, HID], F32, kind="ExternalInput"),
        nc.dram_tensor("W2", [HID, HID], F32, kind="ExternalInput"),
        nc.dram_tensor("W3", [HID, HID], F32, kind="ExternalInput"),
    ]
    fcW_d = nc.dram_tensor("fcW", [HID, 1], F32, kind="ExternalInput")
    g_d = [nc.dram_tensor("g1", [HID, 1], F32, kind="ExternalInput"),
           nc.dram_tensor("g2", [HID, 1], F32, kind="ExternalInput")]
    bt_d = [nc.dram_tensor("bt1", [HID, 1], F32, kind="ExternalInput"),
            nc.dram_tensor("bt2", [HID, 1], F32, kind="ExternalInput")]
    b3_d = nc.dram_tensor("b3", [HID, 1], F32, kind="ExternalInput")
    outv = nc.dram_tensor("outv", [1, NPC], F32, kind="ExternalOutput")

    with tile.TileContext(nc) as tc:
        with (
            tc.tile_pool(name="meta", bufs=1) as meta,
            tc.tile_pool(name="msgp", bufs=8) as msgp,
            tc.tile_pool(name="maskp", bufs=8) as maskp,
            tc.tile_pool(name="zsp", bufs=3) as zsp,
            tc.tile_pool(name="actp", bufs=3) as actp,
            tc.tile_pool(name="hp", bufs=3) as hp,
            tc.tile_pool(name="sqp", bufs=2) as sqp,
            tc.tile_pool(name="zps_p", bufs=2, space="PSUM") as zps_p,
            tc.tile_pool(name="yps_p", bufs=2, space="PSUM") as yps_p,
            tc.tile_pool(name="trps_p", bufs=2, space="PSUM") as trps_p,
            tc.tile_pool(name="fcps_p", bufs=1, space="PSUM") as fcps_p,
            tc.tile_pool(name="dram", bufs=1, space="DRAM") as dram,
        ):
            # ---- resident metadata ----
            gidx_sb = meta.tile([P, T], I32)
            nc.sync.dma_start(gidx_sb[:], gidx_d[:])
            dloc_sb = meta.tile([P, T], F32)
            nc.sync.dma_start(dloc_sb[:], dloc_d[:])
            dinv_sl = meta.tile([P, NT], F32)
            nc.sync.dma_start(dinv_sl[:], dinv_d[:])
            W_sb = []
            for l in range(3):
                fi = IN if l == 0 else HID
                w = meta.tile([fi, HID], F32, name=f"W{l}_sb")
                nc.sync.dma_start(w[:], W_d[l][:])
                W_sb.append(w)
            fcW_sb = meta.tile([HID, 1], F32)
            nc.sync.dma_start(fcW_sb[:], fcW_d[:])
            g_sb, bt_sb = [], []
            for l in range(2):
                gg = meta.tile([HID, 1], F32, name=f"g{l}_sb")
                nc.sync.dma_start(gg[:], g_d[l][:])
                g_sb.append(gg)
                bb = meta.tile([HID, 1], F32, name=f"bt{l}_sb")
                nc.sync.dma_start(bb[:], bt_d[l][:])
                bt_sb.append(bb)
            b3_sb = meta.tile([HID, 1], F32)
            nc.sync.dma_start(b3_sb[:], b3_d[:])
            eps_sb = meta.tile([P, 1], F32)
            nc.vector.memset(eps_sb[:], BN_EPS)

            ident = meta.tile([P, P], F32)
            make_identity(nc, ident[:])
            iota_i = meta.tile([P, P], I32)
            nc.gpsimd.iota(iota_i[:], pattern=[[1, P]], base=0, channel_multiplier=0)
            iota_t = meta.tile([P, P], DT_TAB)
            nc.vector.tensor_copy(iota_t[:], iota_i[:])

            # dinv broadcast rows: dinv_bc[:, t*128+j] = dinv of node t*128+j (all partitions)
            dinv_bc = meta.tile([P, NT * P], F32)
            for t in range(NT):
                tr = trps_p.tile([P, P], F32, tag="tr")
                nc.tensor.transpose(tr[:], dinv_sl[:, t:t + 1].to_broadcast([P, P]), ident[:])
                nc.vector.tensor_copy(dinv_bc[:, t * P:(t + 1) * P], tr[:])

            ystore = meta.tile([P, NT * P], F32)
            sums = meta.tile([P, NT], F32)
            sumsq = meta.tile([P, NT], F32)
            out_store = meta.tile([1, NPC], F32)

            # internal DRAM for collectives
            tab_in = [dram.tile([NPC, HID], DT_TAB, name=f"tab{l}_in") for l in (1, 2)]
            tab_out = [dram.tile([N, HID], DT_TAB, name=f"tab{l}_out", addr_space="Shared")
                       for l in (1, 2)]
            st_in = [dram.tile([P, 2], F32, name=f"st{l}_in") for l in (0, 1)]
            st_out = [dram.tile([P, 2], F32, name=f"st{l}_out", addr_space="Shared")
                      for l in (0, 1)]

            off = [0]
            for t in range(NT):
                off.append(off[-1] + S[t])

            n_layers = 1 if STAGE <= 3 else (2 if STAGE <= 5 else 3)
            for l in range(n_layers):
                f_in = IN if l == 0 else HID
                table = xs if l == 0 else tab_out[l - 1]
                # ---- aggregation + weight matmul ----
                for t in range(NT):
                    d_hi = LAST_D if t == NT - 1 else P
                    zps = zps_p.tile([P, P], F32, tag="zps")
                    for s in range(S[t]):
                        g = off[t] + s
                        msg = msgp.tile([P, f_in], DT_TAB, tag="msg")
                        nc.gpsimd.indirect_dma_start(
                            out=msg[:], out_offset=None, in_=table[:],
                            in_offset=bass.IndirectOffsetOnAxis(ap=gidx_sb[:, g:g + 1], axis=0),
                        )
                        mask = maskp.tile([P, P], DT_TAB, tag="mask")
                        nc.vector.tensor_scalar(
                            out=mask[:], in0=iota_t[:], scalar1=dloc_sb[:, g:g + 1],
                            scalar2=None, op0=Alu.is_equal,
                        )
                        nc.tensor.matmul(zps[:f_in, :], lhsT=msg[:], rhs=mask[:],
                                         start=(s == 0), stop=(s == S[t] - 1))
                    zs = zsp.tile([P, P], F32, tag="zs")
                    nc.vector.tensor_tensor(
                        out=zs[:f_in, :], in0=zps[:f_in, :],
                        in1=dinv_bc[:f_in, t * P:(t + 1) * P], op=Alu.mult,
                    )
                    yps = yps_p.tile([P, P], F32, tag="yps")
                    nc.tensor.matmul(yps[:], lhsT=W_sb[l][:], rhs=zs[:f_in, :],
                                     start=True, stop=True)
                    if l < 2:
                        nc.scalar.activation(
                            out=ystore[:, t * P:t * P + d_hi], in_=yps[:, :d_hi],
                            func=Act.Copy, accum_out=sums[:, t:t + 1],
                        )
                        sq = sqp.tile([P, P], F32, tag="sq")
                        nc.scalar.activation(
                            out=sq[:, :d_hi], in_=yps[:, :d_hi],
                            func=Act.Square, accum_out=sumsq[:, t:t + 1],
                        )
                    else:
                        act3 = actp.tile([P, P], F32, tag="act")
                        nc.scalar.activation(out=act3[:, :d_hi], in_=yps[:, :d_hi],
                                             func=Act.Relu, bias=b3_sb[:], scale=1.0)
                        fcp = fcps_p.tile([1, P], F32, tag="fcp")
                        nc.tensor.matmul(fcp[:1, :d_hi], lhsT=fcW_sb[:], rhs=act3[:, :d_hi],
                                         start=True, stop=True)
                        nc.vector.tensor_copy(out_store[:1, t * P:t * P + d_hi], fcp[:1, :d_hi])

                if STAGE == 1:
                    nc.vector.tensor_copy(out_store[:1, :P], ystore[:1, :P])
                    break
                if l < 2:
                    # ---- BN stats allreduce + coefficients ----
                    stats = meta.tile([P, 2], F32, name=f"stats{l}")
                    nc.vector.tensor_reduce(stats[:, 0:1], sums[:], axis=mybir.AxisListType.X, op=Alu.add)
                    nc.vector.tensor_reduce(stats[:, 1:2], sumsq[:], axis=mybir.AxisListType.X, op=Alu.add)
                    nc.sync.dma_start(st_in[l][:], stats[:])
                    nc.gpsimd.collective_compute(
                        "AllReduce", Alu.add, replica_groups=[list(range(NCORES))],
                        ins=[st_in[l][:]], outs=[st_out[l][:]],
                    )
                    tot = meta.tile([P, 2], F32, name=f"tot{l}")
                    nc.sync.dma_start(tot[:], st_out[l][:])
                    cf = meta.tile([P, 6], F32, name=f"cf{l}")  # mean ex2 var std A B
                    nc.vector.tensor_scalar_mul(cf[:, 0:1], tot[:, 0:1], 1.0 / N)
                    nc.vector.tensor_scalar_mul(cf[:, 1:2], tot[:, 1:2], 1.0 / N)
                    nc.vector.tensor_tensor(out=cf[:, 2:3], in0=cf[:, 0:1], in1=cf[:, 0:1], op=Alu.mult)
                    nc.vector.tensor_tensor(out=cf[:, 2:3], in0=cf[:, 1:2], in1=cf[:, 2:3], op=Alu.subtract)
                    nc.scalar.activation(out=cf[:, 3:4], in_=cf[:, 2:3], func=Act.Sqrt, bias=eps_sb[:], scale=1.0)
                    nc.vector.reciprocal(cf[:, 4:5], cf[:, 3:4])
                    A = meta.tile([P, 1], F32, name=f"A{l}")
                    B = meta.tile([P, 1], F32, name=f"B{l}")
                    nc.vector.tensor_tensor(out=A[:], in0=g_sb[l][:], in1=cf[:, 4:5], op=Alu.mult)
                    nc.vector.tensor_tensor(out=cf[:, 5:6], in0=cf[:, 0:1], in1=A[:], op=Alu.mult)
                    nc.vector.tensor_tensor(out=B[:], in0=bt_sb[l][:], in1=cf[:, 5:6], op=Alu.subtract)
                    if STAGE == 2:
                        nc.vector.tensor_copy(out_store[:1, 0:1], B[:1, :])
                        break

                    # ---- epilogue: act, transpose to node-major, store table slice ----
                    for t in range(NT):
                        d_hi = LAST_D if t == NT - 1 else P
                        act = actp.tile([P, P], F32, tag="act")
                        nc.scalar.activation(out=act[:, :d_hi], in_=ystore[:, t * P:t * P + d_hi],
                                             func=Act.Relu, bias=B[:], scale=A[:])
                        tr = trps_p.tile([P, P], F32, tag="tr")
                        nc.tensor.transpose(tr[:d_hi, :], act[:, :d_hi], ident[:])
                        h = hp.tile([P, HID], DT_TAB, tag="h")
                        nc.vector.tensor_scalar_mul(h[:d_hi, :], tr[:d_hi, :], dinv_sl[:d_hi, t:t + 1])
                        nc.sync.dma_start(tab_in[l][t * P:t * P + d_hi, :], h[:d_hi, :])
                    nc.gpsimd.collective_compute(
                        "AllGather", Alu.bypass, replica_groups=[list(range(NCORES))],
                        ins=[tab_in[l][:]], outs=[tab_out[l][:]],
                    )
                    if STAGE == 3 and l == 0:
                        hh = hp.tile([P, HID], DT_TAB, tag="h")
                        nc.sync.dma_start(hh[:], tab_out[0][:P, :])
                        nc.vector.tensor_copy(out_store[:1, :P], hh[:1, :])
                        break
                    if STAGE == 4 and l == 1:
                        nc.vector.tensor_copy(out_store[:1, :P], ystore[:1, :P])
                        break

            nc.sync.dma_start(outv[:], out_store[:])

    nc.compile()
    return nc


def _prep(inputs):
    x = np.asarray(inputs["x"], np.float32)
    ei = np.asarray(inputs["edge_index"], np.int64)
    loops = np.arange(N, dtype=np.int64)
    src = np.concatenate([ei[0], loops])
    dst = np.concatenate([ei[1], loops])
    deg = np.bincount(dst, minlength=N).astype(np.float32)
    dinv = (1.0 / np.sqrt(deg)).astype(np.float32)
    xs = (x * dinv[:, None]).astype(NP_TAB)

    core = dst // NPC
    rem = dst - core * NPC
    tidx = rem >> 7
    order = np.lexsort((tidx, core))
    src_s = src[order].astype(np.int32)
    core_s = core[order]
    tidx_s = tidx[order]
    loc_s = (rem[order] & 127).astype(np.float32)

    gk = core_s * NT + tidx_s
    cnt = np.bincount(gk, minlength=NCORES * NT).reshape(NCORES, NT)
    S = np.maximum(np.ceil(cnt.max(axis=0) / P).astype(np.int64), 1)
    T = int(S.sum())
    off = np.zeros(NT, np.int64)
    off[1:] = np.cumsum(S)[:-1]

    starts = np.zeros(NCORES * NT, np.int64)
    starts[1:] = np.cumsum(cnt.reshape(-1))[:-1]
    pos = np.arange(len(src_s)) - starts[gk]
    sub = pos >> 7
    lane = pos & 127
    col = off[tidx_s] + sub

    gidx = np.zeros((NCORES, P, T), np.int32)
    dloc = np.full((NCORES, P, T), 1000.0, np.float32)
    gidx[core_s, lane, col] = src_s
    dloc[core_s, lane, col] = loc_s

    dinv_pad = np.zeros(NCORES * NT * P, np.float32)
    dv = dinv.reshape(NCORES, NPC)
    dinv_pad = np.zeros((NCORES, NT * P), np.float32)
    dinv_pad[:, :NPC] = dv
    dinv_sl = dinv_pad.reshape(NCORES, NT, P).transpose(0, 2, 1).copy()  # [c, P, NT]

    com = {
        "xs": np.ascontiguousarray(xs),
        "W1": np.asarray(inputs["W1"], np.float32),
        "W2": np.asarray(inputs["W2"], np.float32),
        "W3": np.asarray(inputs["W3"], np.float32),
        "fcW": np.asarray(inputs["fcW"], np.float32).reshape(HID, 1),
        "g1": np.asarray(inputs["g1"], np.float32).reshape(HID, 1),
        "g2": np.asarray(inputs["g2"], np.float32).reshape(HID, 1),
        "bt1": np.asarray(inputs["bt1"], np.float32).reshape(HID, 1),
        "bt2": np.asarray(inputs["bt2"], np.float32).reshape(HID, 1),
        "b3": np.asarray(inputs["b3"], np.float32).reshape(HID, 1),
    }
    in_maps = []
    for c in range(NCORES):
        m = dict(com)
        m["gidx"] = np.ascontiguousarray(gidx[c])
        m["dloc"] = np.ascontiguousarray(dloc[c])
        m["dinv_sl"] = np.ascontiguousarray(dinv_sl[c])
        in_maps.append(m)
    return in_maps, T, tuple(int(s) for s in S)


def _get_nc(T, S):
    key = (T, S, BF16_TABLES, STAGE)
    if key not in _NC_CACHE:
        _NC_CACHE[key] = _build(T, list(S))
    return _NC_CACHE[key]


def _run(in_maps, T, S):
    nc = _get_nc(T, S)
    r = bass_utils.run_bass_kernel_spmd(nc, in_maps, core_ids=list(range(NCORES)), trace=False)
    return r


def kernel(**inputs):
    in_maps, T, S = _prep(inputs)
    r = _run(in_maps, T, S)
    out = np.concatenate([r.results[c]["outv"].reshape(-1) for c in range(NCORES)])
    fcb = np.asarray(inputs["fcb"], np.float32).reshape(-1)
    out = (out + fcb[0]).astype(np.float32)[:, None]
    # numerically stable sigmoid in fp32
    sig = np.empty_like(out)
    pos = out >= 0
    sig[pos] = 1.0 / (1.0 + np.exp(-out[pos], dtype=np.float32))
    ex = np.exp(out[~pos], dtype=np.float32)
    sig[~pos] = ex / (1.0 + ex)
    return out, sig
